# revision 37
# baseline (speedup 1.0000x reference)
"""AttentionBlock (GroupNorm -> MHA -> out-proj -> residual) on 8 TRN2
NeuronCores: fp8-DoubleRow implementation, v6.

Sharding: pure data-parallel over batch (B=16) - 2 batch elements per core,
no collectives; each core runs the identical program on its own x shard.

v6 structure (per core, 2 batch elements):
  - x ships TWICE from host: fp8 x8 (the only form compute consumes: QKV
    projections are fp8-DoubleRow on raw x8 with the GroupNorm affine folded
    into every PSUM evacuation) and f32r x (residual only, streamed during
    the attention phase, off the critical path). Head latency is set by
    x8(b0) + w8[QK] DMA (~4.9us) instead of f32 x (~7.8us).
  - GroupNorm stats from x8: b0 via DVE bn_stats in the head window; b1
    entirely on gpsimd (square + XYZWC reduce) which idles mid-stream.
  - K evacuations are pre-scaled by A*SCALE*rstd (A = 8*log2 e), so PSUM
    scores arrive as A*logit: ACT chunks exp with scale=1/A; offloaded
    "fast-exp" chunks skip ACT entirely: a K=1 ones-row matmul adds B to
    the PSUM scores and one DVE tensor_scalar (max 0, min 126 -> int8)
    writes the fp8 bit pattern of ~exp(logit)*2^((B-56)/8) directly
    (Schraudolph in the fp8 domain). Offload is whole-(b,h,ch)-chunk so the
    decode's constant factor cancels in softmax normalization.
  - row sums via ones fp8-DR matmul; on = av * (1/row) on DVE; out-proj
    fp8-DR; res evac fuses +bias +residual (STT from f32r x).
"""
import sys

sys.path.insert(0, "/opt/trn_rl_repo")

import numpy as np
import ml_dtypes

import concourse.bass as bass
import concourse.bass_isa as bass_isa
import concourse.mybir as mybir
import concourse.tile as tile
from concourse import bacc
from concourse.bass_utils import run_bass_kernel_spmd

F32 = mybir.dt.float32
F32R = mybir.dt.float32r
F8 = mybir.dt.float8e4
I8 = mybir.dt.int8
AX = mybir.AxisListType
OP = mybir.AluOpType
ACT = mybir.ActivationFunctionType
DR = mybir.MatmulPerfMode.DoubleRow

N_CORES = 8
B, C, H, W = 16, 512, 32, 32
S = H * W                     # 1024
NH, HD = 4, C // 4            # 4 heads x 128
BPC = B // N_CORES            # 2 batch elements per core
CT = C // 128                 # 4 channel tiles
ST = S // 128                 # 8 sequence tiles
NP = ST // 2                  # 4 sequence-tile pairs
EPS = 1e-5
SCALE = 1.0 / float(np.sqrt(HD))
N_ELEM = float(C * S)
A_FE = 8.0 * 1.4426950408889634      # 8*log2(e): fp8-domain exp slope
B_FE = 45.4                          # calibrated fp8-domain exp offset

DEFAULT_CFG = {
    "xload_bufs": 8, "x8_bufs": 4, "qk_bufs": 14,
    "vt_bufs": 8, "et_bufs": 13, "on_bufs": 4, "res_bufs": 6,
    "rbc_bufs": 3,
    "big_bufs": 3, "sm_bufs": 1, "row_bufs": 1,
    "warmup_mms": 8, "sc_prio": 30, "exp_prio": 0,
    "use_fastexp": True,
    # (b, h, ch) chunks offloaded to the DVE fast-exp path
    "fast_chunks": ((0, 2, 0),),
    # engine assignment of the Q/K PSUM evacuations, per batch: m-tile
    # indices listed go to ACT instead of DVE
    "qk_evac_act": {0: (0,), 1: (3, 7)},
    "vt_evac_act": {0: (), 1: ()},
    "tail_act": (0, 2),
    "tail_split": False,
    "stats1_pool": True,
}


def build_program_v6(cfg: dict | None = None) -> bass.Bass:
    cfg = {**DEFAULT_CFG, **(cfg or {})}
    nc = bacc.Bacc()
    x8_d = nc.dram_tensor("x8", [BPC, 2, 128, 2, S], F8, kind="ExternalInput")
    x_d = nc.dram_tensor("x", [BPC, C, S], F32R, kind="ExternalInput")
    w8_d = nc.dram_tensor("w8", [2, 128, 2, 3 * C], F8, kind="ExternalInput")
    wo8_d = nc.dram_tensor("wo8", [2, 128, 2, C], F8, kind="ExternalInput")
    # host-packed consts: one small blob + one broadcast blob (fewer HWDGE
    # generation slots: 625ns each, serialized, on the first-exp path)
    csm_d = nc.dram_tensor("csm", [128, 28], F32, kind="ExternalInput")
    cbc_d = nc.dram_tensor("cbc", [128, 2 * C], F32, kind="ExternalInput")
    eye_d = nc.dram_tensor("eye", [128, 128], F32R, kind="ExternalInput")
    y_d = nc.dram_tensor("y", [BPC, C, S], F32, kind="ExternalOutput")

    use_fe = cfg["use_fastexp"]
    fast_chunks = set(cfg["fast_chunks"]) if use_fe else set()

    with tile.TileContext(nc) as tc:
        with (
            tc.tile_pool(name="const", bufs=1) as cpool,
            tc.tile_pool(name="sb", bufs=1) as sb,
            tc.tile_pool(name="ps", bufs=1, space="PSUM") as ps,
        ):
            # ---- constant tiles ----
            w8 = [cpool.tile([128, 2, 3 * C], F8, name=f"w8_{blk}")
                  for blk in range(2)]
            wo8 = [cpool.tile([128, 2, C], F8, name=f"wo8_{blk}")
                   for blk in range(2)]
            csm = cpool.tile([128, 28], F32, name="csm")
            bqkv_t = csm[:, 0:12]
            wsum_t = csm[:, 12:24]
            bout_t = csm[:, 24:28]
            cbc = cpool.tile([128, 2 * C], F32, name="cbc")
            bv_bc = cbc[:, 0:C]
            wsv_bc = cbc[:, C:2 * C]
            eye_t = cpool.tile([128, 128], F32R, name="eye_t")
            # const building on gpsimd: keeps DVE free for b0 bn_stats.
            # wu_t first: PE warmups gate on it.
            wu_t32 = cpool.tile([128, 512], F32, name="wu_t32")
            nc.gpsimd.memset(wu_t32, 0.001)
            wu_t = cpool.tile([128, 512], F32R, name="wu_t")
            nc.gpsimd.tensor_copy(out=wu_t, in_=wu_t32)
            ones32 = cpool.tile([128, 256], F32, name="ones32")
            nc.gpsimd.memset(ones32, 1.0)
            ones8 = cpool.tile([128, 2, 128], F8, name="ones8")
            nc.gpsimd.tensor_copy(out=ones8, in_=ones32)
            ones128 = cpool.tile([128, 128], F32R, name="ones128")
            nc.gpsimd.tensor_copy(out=ones128, in_=ones32[:, 0:128])
            onesq32 = cpool.tile([1, 512], F32, name="onesq32")
            nc.gpsimd.memset(onesq32, 1.0)
            onesq = cpool.tile([1, 512], F32R, name="onesq")
            nc.gpsimd.tensor_copy(out=onesq, in_=onesq32)
            nbias = cpool.tile([128, 1], F32, name="nbias")
            nc.vector.memset(nbias, -3.0)
            brow32 = cpool.tile([1, 128], F32, name="brow32")
            nc.gpsimd.memset(brow32, B_FE)
            brow = cpool.tile([1, 128], F32R, name="brow")
            nc.gpsimd.tensor_copy(out=brow, in_=brow32)
            # dummy tiny exp: hoists the ACT table load into the idle head
            tldum = cpool.tile([128, 1], F32, name="tldum")
            nc.scalar.activation(out=tldum, in_=nbias, func=ACT.Exp)

            def load_x8(b):
                """Per-sub DMAs so b0 bn_stats can start on the first
                quarter-tile's arrival."""
                x8 = [sb.tile([128, 2, S], F8, tag="x8", bufs=cfg["x8_bufs"],
                              name=f"x8_{b}_{blk}") for blk in range(2)]
                for blk in range(2):
                    for sub in range(2):
                        nc.sync.dma_start(out=x8[blk][:, sub, :],
                                          in_=x8_d[b, blk, :, sub, :])
                return x8

            def load_w8_qk():
                for blk in range(2):
                    nc.sync.dma_start(out=w8[blk][:, :, 0:2 * C],
                                      in_=w8_d[blk][:, :, 0:2 * C])

            def load_csmall():
                nc.sync.dma_start(out=csm, in_=csm_d[:, :])
                nc.sync.dma_start(out=cbc, in_=cbc_d[:, :])

            def load_w8_v():
                for blk in range(2):
                    nc.sync.dma_start(out=w8[blk][:, :, 2 * C:3 * C],
                                      in_=w8_d[blk][:, :, 2 * C:3 * C])

            def load_consts():
                nc.sync.dma_start(out=wo8[0], in_=wo8_d[0])
                nc.sync.dma_start(out=wo8[1], in_=wo8_d[1])
                nc.sync.dma_start(out=eye_t, in_=eye_d[:, :])

            def load_x(b):
                xts = []
                for t in range(CT):
                    xt = sb.tile([128, S], F32R, tag="xload",
                                 bufs=cfg["xload_bufs"], name=f"x{b}_{t}")
                    nc.sync.dma_start(out=xt, in_=x_d[b, t * 128:(t + 1) * 128, :])
                    xts.append(xt)
                return xts

            def finish_stats(b, red):
                """red: [128, 2] replicated (sum_x, sum_x2). Returns
                (scal, dneg, scalA, dnegA, dv_bc)."""
                inv = 1.0 / N_ELEM
                scal = sb.tile([128, 6], F32, tag="scal", bufs=2, name=f"scal{b}")
                # cols: 0=mean 1=rstd 2=v 3=tmp 4=A*SCALE*rstd 5=-mean*rstd
                nc.vector.tensor_scalar_mul(scal[:, 0:1], red[:, 0:1], inv)
                nc.vector.tensor_scalar_mul(scal[:, 3:4], red[:, 1:2], inv)
                nc.vector.scalar_tensor_tensor(
                    out=scal[:, 2:3], in0=scal[:, 0:1], scalar=scal[:, 0:1],
                    in1=scal[:, 3:4], op0=OP.mult, op1=OP.subtract)
                nc.vector.tensor_scalar(scal[:, 2:3], scal[:, 2:3], -1.0, EPS,
                                        op0=OP.mult, op1=OP.add)
                # rstd = 1/sqrt(v) by one Newton step from y0=1/v
                nc.vector.reciprocal(out=scal[:, 1:2], in_=scal[:, 2:3])
                nc.vector.scalar_tensor_tensor(
                    out=scal[:, 3:4], in0=scal[:, 1:2], scalar=scal[:, 1:2],
                    in1=scal[:, 2:3], op0=OP.mult, op1=OP.mult)
                nc.vector.tensor_scalar(scal[:, 3:4], scal[:, 3:4], -0.5, 1.5,
                                        op0=OP.mult, op1=OP.add)
                nc.vector.tensor_tensor(out=scal[:, 1:2], in0=scal[:, 1:2],
                                        in1=scal[:, 3:4], op=OP.mult)
                nc.vector.tensor_scalar_mul(scal[:, 4:5], scal[:, 1:2],
                                            A_FE * SCALE)
                # -mean*rstd (for dneg);  dneg = bqkv - mean*rstd*wsum
                nc.vector.tensor_scalar(scal[:, 5:6], scal[:, 0:1],
                                        scal[:, 1:2], -1.0,
                                        op0=OP.mult, op1=OP.mult)
                dneg = sb.tile([128, 12], F32, tag="dneg", bufs=2,
                               name=f"dneg{b}")
                nc.vector.scalar_tensor_tensor(
                    out=dneg, in0=wsum_t, scalar=scal[:, 5:6], in1=bqkv_t,
                    op0=OP.mult, op1=OP.add)
                # K columns pre-scaled by A*SCALE
                dnegA = sb.tile([128, 4], F32, tag="dnegA", bufs=2,
                                name=f"dnegA{b}")
                nc.vector.tensor_scalar_mul(dnegA, dneg[:, 4:8], A_FE * SCALE)
                # V bias per-channel broadcast: dv = wsv_bc*(-mu*rstd) + bv_bc
                dv = sb.tile([128, C], F32, tag="dv", bufs=2, name=f"dv{b}")
                nc.vector.scalar_tensor_tensor(
                    out=dv, in0=wsv_bc, scalar=scal[:, 5:6], in1=bv_bc,
                    op0=OP.mult, op1=OP.add)
                return scal, dneg, dnegA, dv

            def stats0_pre(x8):
                """b0 stats, DVE part (head window): bn_stats per half."""
                bnb = sb.tile([128, 8, 6], F32, tag="bnb", bufs=1, name="bnb0")
                for blk in range(2):
                    for sub in range(2):
                        for hf in range(2):
                            nc.vector.bn_stats(
                                out=bnb[:, 4 * blk + 2 * sub + hf:
                                        4 * blk + 2 * sub + hf + 1, :],
                                in_=x8[blk][:, sub, hf * 512:(hf + 1) * 512])
                mv = sb.tile([128, 4], F32, tag="mv", bufs=1, name="mv0")
                nc.vector.bn_aggr(out=mv[:, 0:2], in_=bnb)
                # mv[:,2] = mean^2 + var = E[x^2] per partition
                nc.vector.scalar_tensor_tensor(
                    out=mv[:, 2:3], in0=mv[:, 0:1], scalar=mv[:, 0:1],
                    in1=mv[:, 1:2], op0=OP.mult, op1=OP.add)
                pr = sb.tile([128, 2], F32R, tag="partr", bufs=1, name="pr0")
                nc.vector.tensor_copy(out=pr[:, 0:1], in_=mv[:, 0:1])
                nc.vector.tensor_copy(out=pr[:, 1:2], in_=mv[:, 2:3])
                return pr

            def stats0_post(pr):
                """Cross-partition combine on an idle-PE ones-matmul (the
                gpsimd all-reduce would queue behind b1's Pool stats);
                emitted AFTER the first QK matmuls so it doesn't head-block
                the in-order PE stream."""
                red_ps = ps.tile([128, 512], F32, tag="row",
                                 bufs=cfg["row_bufs"], name="red0_ps")
                nc.tensor.matmul(red_ps[:, 0:2], ones128, pr,
                                 start=True, stop=True)
                red = sb.tile([128, 2], F32, tag="tsb", bufs=2, name="red0")
                # per-partition means summed over 128 partitions -> scale to
                # full-population sums so finish_stats' 1/N works unchanged
                nc.vector.tensor_scalar(red, red_ps[:, 0:2], N_ELEM / 128.0,
                                        0.0, op0=OP.mult, op1=OP.add)
                return finish_stats(0, red)

            def stats1_pool(x8):
                """b1 stats entirely on gpsimd (SBUF-only)."""
                if not cfg["stats1_pool"]:
                    bnb = sb.tile([128, 8, 6], F32, tag="bnb", bufs=1,
                                  name="bnb1")
                    for blk in range(2):
                        for sub in range(2):
                            for hf in range(2):
                                nc.vector.bn_stats(
                                    out=bnb[:, 4 * blk + 2 * sub + hf:
                                            4 * blk + 2 * sub + hf + 1, :],
                                    in_=x8[blk][:, sub, hf * 512:(hf + 1) * 512])
                    mv = sb.tile([128, 4], F32, tag="mv", bufs=1, name="mv1")
                    nc.vector.bn_aggr(out=mv[:, 0:2], in_=bnb)
                    nc.vector.scalar_tensor_tensor(
                        out=mv[:, 2:3], in0=mv[:, 0:1], scalar=mv[:, 0:1],
                        in1=mv[:, 1:2], op0=OP.mult, op1=OP.add)
                    pr = sb.tile([128, 2], F32, tag="partr", bufs=1, name="pr1")
                    nc.vector.tensor_copy(out=pr[:, 0:1], in_=mv[:, 0:1])
                    nc.vector.tensor_copy(out=pr[:, 1:2], in_=mv[:, 2:3])
                    red = sb.tile([128, 2], F32, tag="tsb", bufs=2, name="red1")
                    nc.gpsimd.partition_all_reduce(red, pr, 128,
                                                   bass_isa.ReduceOp.add)
                    nc.vector.tensor_scalar_mul(red, red, N_ELEM / 128.0)
                    return finish_stats(1, red)
                sq = sb.tile([128, 2, S], F32, tag="sq1", bufs=2, name="sq1")
                gs1 = sb.tile([1, 4], F32, tag="gs1", bufs=1, name="gs1")
                for blk in range(2):
                    nc.gpsimd.tensor_reduce(out=gs1[:, blk:blk + 1],
                                            in_=x8[blk], op=OP.add,
                                            axis=AX.XYZWC)
                    nc.gpsimd.tensor_tensor(out=sq, in0=x8[blk], in1=x8[blk],
                                            op=OP.mult)
                    nc.gpsimd.tensor_reduce(out=gs1[:, 2 + blk:3 + blk],
                                            in_=sq, op=OP.add, axis=AX.XYZWC)
                gbc = sb.tile([128, 4], F32, tag="gbc", bufs=1, name="gbc1")
                nc.gpsimd.partition_broadcast(gbc, gs1, 128)
                red = sb.tile([128, 2], F32, tag="tsb", bufs=2, name="red1")
                nc.vector.tensor_tensor(out=red[:, 0:1], in0=gbc[:, 0:1],
                                        in1=gbc[:, 1:2], op=OP.add)
                nc.vector.tensor_tensor(out=red[:, 1:2], in0=gbc[:, 2:3],
                                        in1=gbc[:, 3:4], op=OP.add)
                return finish_stats(1, red)

            def qk_mm(b, m, x8):
                mm = ps.tile([128, S], F32, tag="big", bufs=cfg["big_bufs"],
                             name=f"mmq{b}_{m}")
                for ch in range(2):
                    for blk in range(2):
                        nc.tensor.matmul(
                            mm[:, ch * 512:(ch + 1) * 512],
                            w8[blk][:, :, m * 128:(m + 1) * 128],
                            x8[blk][:, :, ch * 512:(ch + 1) * 512],
                            start=(blk == 0), stop=(blk == 1), perf_mode=DR)
                return mm

            def qk_evac(b, m, mm, st):
                scal, dneg, dnegA, _ = st
                qt = sb.tile([128, S], F32R, tag="qk", bufs=cfg["qk_bufs"],
                             name=f"qk{b}_{m}")
                if m >= 4:
                    sc_ap, bias_ap = scal[:, 4:5], dnegA[:, m - 4:m - 3]
                else:
                    sc_ap, bias_ap = scal[:, 1:2], dneg[:, m:m + 1]
                if m in cfg["qk_evac_act"][b]:
                    nc.scalar.activation(out=qt, in_=mm, func=ACT.Identity,
                                         scale=sc_ap, bias=bias_ap)
                else:
                    nc.vector.tensor_scalar(qt, mm, sc_ap, bias_ap,
                                            op0=OP.mult, op1=OP.add)
                return qt

            def qk_mtile(b, m, x8, st):
                """Q (m 0..3) or K (m 4..7) channel-tile: [128, S] f32r.
                K tiles are pre-scaled by A*SCALE (fp8-domain exp slope)."""
                return qk_evac(b, m, qk_mm(b, m, x8), st)

            def vt_pair(b, p, x8, st):
                """V for sequence tiles (2p, 2p+1): fp8 [128, 2, C].
                v = rstd*(W8 @ x8) + dv  (GroupNorm affine folded)."""
                scal, _, _, dv = st
                vt = sb.tile([128, 2, C], F8, tag="vt", bufs=cfg["vt_bufs"],
                             name=f"vt{b}_{p}")
                for i in range(2):
                    stile = 2 * p + i
                    mm = ps.tile([128, 512], F32, tag="sm",
                                 bufs=cfg["sm_bufs"], name=f"mmv{b}_{stile}")
                    for blk in range(2):
                        nc.tensor.matmul(
                            mm, x8[blk][:, :, stile * 128:(stile + 1) * 128],
                            w8[blk][:, :, 2 * C:3 * C],
                            start=(blk == 0), stop=(blk == 1), perf_mode=DR)
                    nc.vector.scalar_tensor_tensor(
                        out=vt[:, i, :], in0=mm, scalar=scal[:, 1:2], in1=dv,
                        op0=OP.mult, op1=OP.add)
                return vt

            def alloc_on(b):
                return [sb.tile([128, 2, S], F8, tag="on", bufs=cfg["on_bufs"],
                                name=f"on{b}_{blk}") for blk in range(2)]

            def attn_scores(b, h, ch, q_t, k_t, mid=None, split_last=False):
                """Score matmuls + exp for one (head, q-half); returns ets.
                Fast chunks skip ACT: +B ones-row matmul, then DVE clamps
                PSUM to [0,126] and writes int8 = the fp8 bit pattern.
                split_last: the final p-tile's exp is emitted as two
                256-q-column halves so the tail reduce can start early."""
                fast = (b, h, ch) in fast_chunks
                ets = []
                boost = cfg["sc_prio"]
                for p in range(NP):
                    if p == 2 and mid is not None:
                        mid()
                    sc = ps.tile([128, S], F32, tag="big", bufs=cfg["big_bufs"],
                                 name=f"sc{b}_{h}_{ch}_{p}")
                    if boost:
                        p_save = tc.cur_priority
                        tc.cur_priority = p_save - boost
                    for i in range(2):
                        stile = 2 * p + i
                        nc.tensor.matmul(sc[:, i * 512:(i + 1) * 512],
                                         k_t[:, stile * 128:(stile + 1) * 128],
                                         q_t[:, ch * 512:(ch + 1) * 512],
                                         start=True, stop=not fast)
                        if fast:
                            nc.tensor.matmul(sc[:, i * 512:(i + 1) * 512],
                                             brow, onesq,
                                             start=False, stop=True)
                    if boost:
                        tc.cur_priority = p_save + (tc.cur_priority
                                                    - (p_save - boost))
                    et = sb.tile([128, 2, 512], F8, tag="et", bufs=cfg["et_bufs"],
                                 name=f"et{b}_{h}_{ch}_{p}")
                    eb = cfg["exp_prio"]
                    if eb:
                        e_save = tc.cur_priority
                        tc.cur_priority = e_save - eb
                    if fast:
                        nc.vector.tensor_scalar(et.bitcast(I8), sc, 0.0, 126.0,
                                                op0=OP.max, op1=OP.min)
                    elif split_last and p == NP - 1:
                        for qh in range(2):
                            nc.scalar.activation(
                                out=et[:, :, qh * 256:(qh + 1) * 256],
                                in_=sc[:, :].rearrange("p (i q) -> p i q", i=2)
                                [:, :, qh * 256:(qh + 1) * 256],
                                func=ACT.Exp, scale=1.0 / A_FE,
                                bias=nbias[:, 0:1])
                    else:
                        # scores arrive as A*logit: exp((1/A)*sc - 3)
                        nc.scalar.activation(out=et, in_=sc, func=ACT.Exp,
                                             scale=1.0 / A_FE,
                                             bias=nbias[:, 0:1])
                    if eb:
                        tc.cur_priority = e_save + 1
                    ets.append(et)
                return ets

            def attn_reduce(b, h, ch, ets, vts, on):
                """Row sums, AV, and softmax normalization for one chunk."""
                row = ps.tile([128, 512], F32, tag="row", bufs=cfg["row_bufs"],
                              name=f"row{b}_{h}_{ch}")
                for p in range(NP):
                    nc.tensor.matmul(row, ones8, ets[p],
                                     start=(p == 0), stop=(p == NP - 1),
                                     perf_mode=DR)
                av = ps.tile([128, 512], F32, tag="sm", bufs=cfg["sm_bufs"],
                             name=f"av{b}_{h}_{ch}")
                for p in range(NP):
                    nc.tensor.matmul(av, vts[p][:, :, h * HD:(h + 1) * HD], ets[p],
                                     start=(p == 0), stop=(p == NP - 1),
                                     perf_mode=DR)
                rbc = sb.tile([128, 512], F32, tag="rbc", bufs=cfg["rbc_bufs"],
                              name=f"rbc{b}_{h}_{ch}")
                nc.vector.reciprocal(out=rbc, in_=row)
                nc.vector.tensor_tensor(
                    out=on[h // 2][:, h % 2, ch * 512:(ch + 1) * 512],
                    in0=av, in1=rbc, op=OP.mult)

            def outproj_m(b, m, on, rx):
                """Full-width out-proj tile m (+bias +residual from rx)."""
                mo = ps.tile([128, S], F32, tag="big", bufs=cfg["big_bufs"],
                             name=f"mo{b}_{m}")
                for ch in range(2):
                    for blk in range(2):
                        nc.tensor.matmul(
                            mo[:, ch * 512:(ch + 1) * 512],
                            wo8[blk][:, :, m * 128:(m + 1) * 128],
                            on[blk][:, :, ch * 512:(ch + 1) * 512],
                            start=(blk == 0), stop=(blk == 1), perf_mode=DR)
                res = sb.tile([128, S], F32, tag="res", bufs=cfg["res_bufs"],
                              name=f"res{b}_{m}")
                nc.vector.scalar_tensor_tensor(
                    out=res, in0=mo, scalar=bout_t[:, m:m + 1], in1=rx,
                    op0=OP.add, op1=OP.add)
                nc.sync.dma_start(out=y_d[b, m * 128:(m + 1) * 128, :], in_=res)

            def outproj_m_ch(b, m, ch, on, rx, res, tag="sm", evac="dve"):
                """Half-width out-proj chunk (m, ch); caller DMAs res."""
                if tag == "big":
                    mo_full = ps.tile([128, S], F32, tag="big",
                                      bufs=cfg["big_bufs"], name=f"mo{b}_{m}_{ch}")
                    mo = mo_full[:, 0:512]
                else:
                    mo = ps.tile([128, 512], F32, tag="sm", bufs=cfg["sm_bufs"],
                                 name=f"mo{b}_{m}_{ch}")
                sl = slice(ch * 512, (ch + 1) * 512)
                for blk in range(2):
                    nc.tensor.matmul(
                        mo, wo8[blk][:, :, m * 128:(m + 1) * 128],
                        on[blk][:, :, ch * 512:(ch + 1) * 512],
                        start=(blk == 0),
                        stop=(blk == 1 and evac != "act"), perf_mode=DR)
                if evac == "act":
                    nc.tensor.matmul(mo, eye_t, rx[:, sl],
                                     start=False, stop=True)
                    nc.scalar.activation(out=res[:, sl], in_=mo,
                                         func=ACT.Identity,
                                         bias=bout_t[:, m:m + 1])
                else:
                    nc.vector.scalar_tensor_tensor(
                        out=res[:, sl], in0=mo, scalar=bout_t[:, m:m + 1],
                        in1=rx[:, sl], op0=OP.add, op1=OP.add)

            # ================= emission schedule =================
            x8_0 = load_x8(0)
            x8_1 = load_x8(1)
            load_w8_qk()
            load_csmall()
            load_w8_v()
            load_consts()
            xts0 = load_x(0)
            xts1 = load_x(1)
            pr0 = stats0_pre(x8_0)
            if cfg["warmup_mms"]:
                n_wu = cfg["warmup_mms"]
                wu_ps = ps.tile([128, 512], F32, tag="sm", bufs=cfg["sm_bufs"],
                                name="wu_ps")
                for i in range(n_wu):
                    nc.tensor.matmul(wu_ps, wu_t[:, 0:128], wu_t,
                                     start=True, stop=True)
            qk0 = {}
            mm00 = qk_mm(0, 0, x8_0)
            mm04 = qk_mm(0, 4, x8_0)
            st0 = stats0_post(pr0)
            qk0[0] = qk_evac(0, 0, mm00, st0)
            qk0[4] = qk_evac(0, 4, mm04, st0)
            vts0 = [vt_pair(0, p, x8_0, st0) for p in range(NP)]
            on0 = alloc_on(0)
            # software-pipelined attention: scores/exp of chunk c+1 are
            # emitted BEFORE reduce (row/av) of chunk c
            e = {}
            e[0] = attn_scores(0, 0, 0, qk0[0], qk0[4])
            qk0[1] = qk_mtile(0, 1, x8_0, st0)
            qk0[5] = qk_mtile(0, 5, x8_0, st0)
            qk1 = {}
            vts1 = []
            e[1] = attn_scores(0, 0, 1, qk0[0], qk0[4],
                               mid=lambda: attn_reduce(0, 0, 0, e[0], vts0, on0))
            qk0[2] = qk_mtile(0, 2, x8_0, st0)
            qk0[6] = qk_mtile(0, 6, x8_0, st0)
            e[2] = attn_scores(0, 1, 0, qk0[1], qk0[5],
                               mid=lambda: attn_reduce(0, 0, 1, e[1], vts0, on0))
            qk0[3] = qk_mtile(0, 3, x8_0, st0)
            qk0[7] = qk_mtile(0, 7, x8_0, st0)
            e[3] = attn_scores(0, 1, 1, qk0[1], qk0[5],
                               mid=lambda: attn_reduce(0, 1, 0, e[2], vts0, on0))
            e[4] = attn_scores(0, 2, 0, qk0[2], qk0[6],
                               mid=lambda: attn_reduce(0, 1, 1, e[3], vts0, on0))
            e[5] = attn_scores(0, 2, 1, qk0[2], qk0[6],
                               mid=lambda: attn_reduce(0, 2, 0, e[4], vts0, on0))
            st1 = stats1_pool(x8_1)
            e[6] = attn_scores(0, 3, 0, qk0[3], qk0[7],
                               mid=lambda: attn_reduce(0, 2, 1, e[5], vts0, on0))
            qk1[0] = qk_mtile(1, 0, x8_1, st1)
            qk1[4] = qk_mtile(1, 4, x8_1, st1)
            e[7] = attn_scores(0, 3, 1, qk0[3], qk0[7],
                               mid=lambda: attn_reduce(0, 3, 0, e[6], vts0, on0))
            qk1[1] = qk_mtile(1, 1, x8_1, st1)
            qk1[5] = qk_mtile(1, 5, x8_1, st1)
            vts1.append(vt_pair(1, 0, x8_1, st1))
            on1 = alloc_on(1)
            res1 = [sb.tile([128, S], F32, tag="res", bufs=cfg["res_bufs"],
                            name=f"res1_{m}") for m in range(CT)]
            d = {}
            d[0] = attn_scores(1, 0, 0, qk1[0], qk1[4])
            attn_reduce(0, 3, 1, e[7], vts0, on0)
            qk1[2] = qk_mtile(1, 2, x8_1, st1)
            qk1[6] = qk_mtile(1, 6, x8_1, st1)
            vts1.append(vt_pair(1, 1, x8_1, st1))
            outproj_m(0, 0, on0, xts0[0])
            d[1] = attn_scores(1, 1, 0, qk1[1], qk1[5])
            vts1.append(vt_pair(1, 2, x8_1, st1))
            vts1.append(vt_pair(1, 3, x8_1, st1))
            attn_reduce(1, 0, 0, d[0], vts1, on1)
            qk1[3] = qk_mtile(1, 3, x8_1, st1)
            qk1[7] = qk_mtile(1, 7, x8_1, st1)
            outproj_m(0, 1, on0, xts0[1])
            d[2] = attn_scores(1, 2, 0, qk1[2], qk1[6])
            attn_reduce(1, 1, 0, d[1], vts1, on1)
            outproj_m(0, 2, on0, xts0[2])
            d[3] = attn_scores(1, 3, 0, qk1[3], qk1[7])
            attn_reduce(1, 2, 0, d[2], vts1, on1)
            outproj_m(0, 3, on0, xts0[3])
            d[4] = attn_scores(1, 0, 1, qk1[0], qk1[4])
            attn_reduce(1, 3, 0, d[3], vts1, on1)
            outproj_m_ch(1, 0, 0, on1, xts1[0], res1[0])
            nc.sync.dma_start(out=y_d[1, 0:128, 0:512], in_=res1[0][:, 0:512])
            d[5] = attn_scores(1, 1, 1, qk1[1], qk1[5])
            attn_reduce(1, 0, 1, d[4], vts1, on1)
            outproj_m_ch(1, 1, 0, on1, xts1[1], res1[1])
            nc.sync.dma_start(out=y_d[1, 128:256, 0:512], in_=res1[1][:, 0:512])
            d[6] = attn_scores(1, 2, 1, qk1[2], qk1[6])
            attn_reduce(1, 1, 1, d[5], vts1, on1)
            outproj_m_ch(1, 2, 0, on1, xts1[2], res1[2])
            nc.sync.dma_start(out=y_d[1, 256:384, 0:512], in_=res1[2][:, 0:512])
            tail_split = cfg["tail_split"]
            d[7] = attn_scores(1, 3, 1, qk1[3], qk1[7], split_last=tail_split)
            attn_reduce(1, 2, 1, d[6], vts1, on1)
            outproj_m_ch(1, 3, 0, on1, xts1[3], res1[3])
            nc.sync.dma_start(out=y_d[1, 384:512, 0:512], in_=res1[3][:, 0:512])
            if not tail_split:
                attn_reduce(1, 3, 1, d[7], vts1, on1)
                for m in range(CT):
                    outproj_m_ch(1, m, 1, on1, xts1[m], res1[m], tag="big",
                                 evac="act" if m in cfg["tail_act"] else "dve")
                    eng = nc.scalar if m % 2 == 0 else nc.sync
                    eng.dma_start(out=y_d[1, m * 128:(m + 1) * 128, 512:1024],
                                  in_=res1[m][:, 512:1024])
            # --- split tail: reduce/outproj/DMA in 256-q-column halves so
            # the drain starts as soon as the first half-exp lands ---
            def tail_half(qh, row_t, av_t, rbc_t):
                qs = slice(qh * 256, (qh + 1) * 256)
                for p in range(NP):
                    nc.tensor.matmul(row_t[:, qs], ones8, d[7][p][:, :, qs],
                                     start=(p == 0), stop=(p == NP - 1),
                                     perf_mode=DR)
                for p in range(NP):
                    nc.tensor.matmul(av_t[:, qs],
                                     vts1[p][:, :, 3 * HD:4 * HD],
                                     d[7][p][:, :, qs],
                                     start=(p == 0), stop=(p == NP - 1),
                                     perf_mode=DR)
                nc.vector.reciprocal(out=rbc_t[:, qs], in_=row_t[:, qs])
                nc.vector.tensor_tensor(
                    out=on1[1][:, 1, 512 + qh * 256:512 + (qh + 1) * 256],
                    in0=av_t[:, qs], in1=rbc_t[:, qs], op=OP.mult)
                for m in range(CT):
                    ys = slice(512 + qh * 256, 512 + (qh + 1) * 256)
                    mo = ps.tile([128, 512], F32, tag="big",
                                 bufs=cfg["big_bufs"], name=f"mot{m}_{qh}")
                    act_ev = m in cfg["tail_act"]
                    for blk in range(2):
                        nc.tensor.matmul(
                            mo[:, 0:256],
                            wo8[blk][:, :, m * 128:(m + 1) * 128],
                            on1[blk][:, :, ys],
                            start=(blk == 0),
                            stop=(blk == 1 and not act_ev), perf_mode=DR)
                    if act_ev:
                        nc.tensor.matmul(mo[:, 0:256], eye_t, xts1[m][:, ys],
                                         start=False, stop=True)
                        nc.scalar.activation(out=res1[m][:, ys],
                                             in_=mo[:, 0:256],
                                             func=ACT.Identity,
                                             bias=bout_t[:, m:m + 1])
                    else:
                        nc.vector.scalar_tensor_tensor(
                            out=res1[m][:, ys], in0=mo[:, 0:256],
                            scalar=bout_t[:, m:m + 1], in1=xts1[m][:, ys],
                            op0=OP.add, op1=OP.add)
                    eng = nc.scalar if m % 2 == 0 else nc.sync
                    eng.dma_start(out=y_d[1, m * 128:(m + 1) * 128, ys],
                                  in_=res1[m][:, ys])

            if tail_split:
                row_t = ps.tile([128, 512], F32, tag="row",
                                bufs=cfg["row_bufs"], name="row_tail")
                av_t = ps.tile([128, 512], F32, tag="sm", bufs=cfg["sm_bufs"],
                                name="av_tail")
                rbc_t = sb.tile([128, 512], F32, tag="rbc",
                                bufs=cfg["rbc_bufs"], name="rbc_tail")
                for qh in range(2):
                    tail_half(qh, row_t, av_t, rbc_t)
    nc.finalize()
    return nc


_cached = {}


def _get_program(cfg=None) -> bass.Bass:
    key = "v6" if cfg is None else repr(sorted((cfg or {}).items()))
    if key not in _cached:
        _cached[key] = build_program_v6(cfg)
    return _cached[key]


def _pack_w8(wT: np.ndarray) -> np.ndarray:
    """[C, N] weight (already transposed, contraction-major) ->
    [2, 128, 2, N] fp8 DoubleRow layout: c = blk*256 + sub*128 + p."""
    n = wT.shape[1]
    return np.ascontiguousarray(
        wT.reshape(2, 2, 128, n).transpose(0, 2, 1, 3)
    ).astype(ml_dtypes.float8_e4m3)


def kernel(x, gn_weight, gn_bias, qkv_w, qkv_b, out_w, out_b):
    x = np.ascontiguousarray(np.asarray(x, dtype=np.float32))
    gn_weight = np.asarray(gn_weight, dtype=np.float32)
    gn_bias = np.asarray(gn_bias, dtype=np.float32)
    qkv_w = np.asarray(qkv_w, dtype=np.float32)
    qkv_b = np.asarray(qkv_b, dtype=np.float32)
    out_w = np.asarray(out_w, dtype=np.float32)
    out_b = np.asarray(out_b, dtype=np.float32)

    # fold the GroupNorm affine into the QKV projection (host-side prep)
    w_eff = qkv_w * gn_weight[None, :]
    b_eff = qkv_b + qkv_w @ gn_bias
    w8 = _pack_w8(np.ascontiguousarray(w_eff.T))       # [2,128,2,3C]
    wo8 = _pack_w8(np.ascontiguousarray(out_w.T))      # [2,128,2,C]
    wsum = w8.astype(np.float32).sum(axis=(0, 1, 2))   # [3C]

    xs = x.reshape(B, C, S)
    # fp8 x in QKV moving layout: [BPC, blk, 128, sub, S], c = blk*256+sub*128+p
    x8_all = np.ascontiguousarray(
        xs.reshape(B, 2, 2, 128, S).transpose(0, 1, 3, 2, 4)
    ).astype(ml_dtypes.float8_e4m3)

    # csm: [128, 28] = bqkv (12 cols) | wsum (12) | bout (4), col m = ch m*128+p
    csm = np.concatenate([
        b_eff.reshape(12, 128).T,
        wsum.reshape(12, 128).T,
        out_b.reshape(4, 128).T,
    ], axis=1).astype(np.float32)
    # cbc: [128, 1024] = bv | wsum_v broadcast across partitions
    cbc = np.broadcast_to(
        np.concatenate([b_eff[2 * C:], wsum[2 * C:]])[None, :], (128, 2 * C)
    ).astype(np.float32)

    nc = _get_program()
    in_maps = []
    for c in range(N_CORES):
        in_maps.append({
            "x": np.ascontiguousarray(xs[c * BPC:(c + 1) * BPC]),
            "x8": np.ascontiguousarray(x8_all[c * BPC:(c + 1) * BPC]),
            "w8": w8,
            "wo8": wo8,
            "csm": np.ascontiguousarray(csm),
            "cbc": np.ascontiguousarray(cbc),
            "eye": np.eye(128, dtype=np.float32),
        })
    r = run_bass_kernel_spmd(nc, in_maps, list(range(N_CORES)))
    out = np.concatenate([r.results[c]["y"] for c in range(N_CORES)], axis=0)
    return out.reshape(B, C, H, W).astype(np.float32)
